# revision 30
# baseline (speedup 1.0000x reference)
"""Multi-head attention (RMSNorm-QK + RoPE) Trainium2 Bass kernel, v2.

Sharding: 8 cores = 4 batches x 2 head-groups (6 heads each).
Each core computes, for its (batch, head-group):
  qkv = x @ Wqkv_slice, rmsnorm+rope on q/k, softmax(q k^T/8) v,
  y_partial = attn_out @ Wproj_rows.
Host sums the two partial y's per batch and adds proj bias.

v2 structure (vs v1):
- Phase 1 produces Q^T/K^T directly (W stationary, x^T moving) -- no PE
  transposes.  RMSNorm sum-of-squares via selector matmuls (partition
  reduce on PE), rsqrt via ScalarE ln/exp, per-seq broadcast via an
  expander matmul.  RoPE rotate-half via a permutation matmul; the
  sign and norm weight are folded into the host cos/sin tables.
- PSUM->SBUF evacuations (squares, copies, V) on ScalarE, elementwise
  rope combines on DVE in bf16.
- Softmax denominators: reciprocal_approx_fast per head (off the
  critical path), replacing full-rate RECIPROCAL + gather copies.
- QKV matmuls in bf16 (x, Wqkv); attention/proj in f32r/bf16.
"""

import sys

for _p in ("/opt/trn_rl_repo", "/root/.axon_site/_ro/trn_rl_repo"):
    if _p not in sys.path:
        sys.path.insert(0, _p)

import numpy as np
import ml_dtypes

import bass_rust
import concourse.bass as bass
import concourse.mybir as mybir
import concourse.tile as tile
from concourse.bass_utils import run_bass_kernel_spmd

# Problem constants (hardcoded per contract)
B, N, D = 4, 2048, 768
H, HD = 12, 64
HPC = 6              # heads per core
NT = N // 128        # 16 seq tiles
NJ = N // 512        # 4 seq chunks
EPS = 1e-6
THETA = 10000.0
SCALE = HD ** -0.5   # 0.125

F32 = mybir.dt.float32
F32R = mybir.dt.float32r
BF16 = mybir.dt.bfloat16
I16 = mybir.dt.int16
ACT = mybir.ActivationFunctionType
ALU = mybir.AluOpType

# Schraudolph exp in bf16 bit-space: i16 = round(s*A16 + B16); bitcast bf16
# approximates exp(SCALE*s).  Verified bit-exact vs host model on HW.
# Key tiles in DVE_KT run their exp on the DVE this way; the rest use
# ScalarE exp.  Softmax renormalization cancels part of the interp error.
A16 = SCALE * 128.0 / np.log(2.0)
B16 = 127.0 * 128.0 - 366393.0 / (1 << 16)
DVE_KT = (2, 5, 8, 11, 14)

KERNEL_TRACE = False
_CACHE = {}


# ---------------------------------------------------------------- wait split
_ctr = [0]


def _mk_nop(engine, waits=None, updates=None):
    _ctr[0] += 1
    si = mybir.SyncInfo(on_wait=waits or [], on_update=updates or [])
    return bass_rust.InstNoOp(
        name=f"I-waitfix-{_ctr[0]}", engine=engine, ins=[], outs=[], sync_info=si
    )


def split_multi_waits(nc):
    """This walrus build accepts only ONE sync wait/update per instruction;
    hoist extras onto adjacent same-engine NoOp carriers."""
    for fn in nc.m.functions:
        for bb in fn.blocks:
            insts = bb.instructions
            out = []
            changed = False
            for inst in insts:
                si = inst.sync_info
                if si is None:
                    out.append(inst)
                    continue
                waits = list(si.on_wait or [])
                updates = list(si.on_update or [])
                pre, post = [], []
                if len(waits) > 1:
                    for w in waits[:-1]:
                        pre.append(_mk_nop(inst.engine, waits=[w]))
                    si.on_wait = [waits[-1]]
                    changed = True
                if len(updates) > 1:
                    if inst.opcode == "DMACopy":
                        raise RuntimeError(
                            f"DMACopy {inst.name} has {len(updates)} updates"
                        )
                    for u in updates[1:]:
                        post.append(_mk_nop(inst.engine, updates=[u]))
                    si.on_update = [updates[0]]
                    changed = True
                out.extend(pre)
                out.append(inst)
                out.extend(post)
            if changed:
                insts[:] = out
    return nc


# ---------------------------------------------------------------- host utils
def round_f32r(a: np.ndarray) -> np.ndarray:
    """Round fp32 -> float32r (RNE to 11 mantissa bits), exact device match."""
    a = np.ascontiguousarray(a, dtype=np.float32)
    b = a.view(np.uint32).astype(np.uint64)
    drop = 12
    half = np.uint64(1 << (drop - 1))
    lsb = (b >> np.uint64(drop)) & np.uint64(1)
    out = (((b + half - np.uint64(1) + lsb) >> np.uint64(drop)) << np.uint64(drop))
    return out.astype(np.uint32).view(np.float32)


def to_bf16(a: np.ndarray) -> np.ndarray:
    return np.ascontiguousarray(a, dtype=np.float32).astype(ml_dtypes.bfloat16)


def _rope_tables_t(norm_w: np.ndarray):
    """Transposed sign-folded tables, duplicated over the two 64-row head
    blocks.  Per head block (d = p % 64):
      q'[d, n] = q[d, n]*cosw[d, n] + q[(d+32)%64, n]*sinw[d, n]
    with cosw[d] = cos[d]*w[d], sinw[d<32] = -sin[d]*w[d+32],
    sinw[d>=32] = sin[d]*w[d-32]."""
    inv_freq = 1.0 / (THETA ** (np.arange(0, HD, 2, dtype=np.float32) / HD))
    t = np.arange(N, dtype=np.float32)
    freqs = np.einsum("i,j->ij", t, inv_freq).astype(np.float32)
    emb = np.concatenate([freqs, freqs], axis=-1)  # [N, HD]
    cos = np.cos(emb).astype(np.float32)
    sin = np.sin(emb).astype(np.float32)
    w = norm_w.astype(np.float32)
    h = HD // 2
    cosw = (cos * w[None, :]).T.copy()            # [HD, N]
    sinw = np.empty((HD, N), np.float32)
    sinw[:h, :] = -(sin[:, :h] * w[None, h:]).T
    sinw[h:, :] = (sin[:, h:] * w[None, :h]).T
    cosd = np.concatenate([cosw, cosw], axis=0)   # [128, N]
    sind = np.concatenate([sinw, sinw], axis=0)
    return to_bf16(cosd), to_bf16(sind)


def _const_mats():
    h = HD // 2
    # perm: out[h0+d] = in[h0+(d+32)%64]  ->  P[p_src, p_out]
    perm = np.zeros((128, 128), np.float32)
    for h0 in (0, 64):
        for d in range(HD):
            perm[h0 + (d + h) % HD, h0 + d] = 1.0
    # sel: [128, 6, 12]  sq partitions -> ss rows
    sel = np.zeros((128, 6, 12), np.float32)
    for g in range(6):
        for p in range(128):
            r = (2 * g if g < 3 else 6 + 2 * (g - 3)) + (1 if p >= 64 else 0)
            sel[p, g, r] = 1.0
    # emat: [12, 6, 128]  ss rows -> broadcast blocks
    emat = np.zeros((12, 6, 128), np.float32)
    for g in range(6):
        for m in range(128):
            r = (2 * g if g < 3 else 6 + 2 * (g - 3)) + (1 if m >= 64 else 0)
            emat[r, g, m] = 1.0
    return to_bf16(perm), to_bf16(sel.reshape(128, 72)), \
        round_f32r(emat.reshape(12, 768))


# ---------------------------------------------------------------- bass build
def build_nc(use_bias: bool, shared_tabs: bool):
    FC = 7 if use_bias else 6  # feature chunks of 128 (7th = bias row)
    nc = bass.Bass()

    xt_d = nc.dram_tensor("xt", [FC * 128, N], BF16, kind="ExternalInput")
    wqk_d = nc.dram_tensor("wqk", [128, FC * 6 * 128], BF16, kind="ExternalInput")
    wv_d = nc.dram_tensor("wv", [FC * 128, HPC * HD], BF16, kind="ExternalInput")
    wo_d = nc.dram_tensor("wo", [HPC * HD, D], F32R, kind="ExternalInput")
    perm_d = nc.dram_tensor("perm", [128, 128], BF16, kind="ExternalInput")
    sel_d = nc.dram_tensor("sel", [128, 72], BF16, kind="ExternalInput")
    emat_d = nc.dram_tensor("emat", [12, 768], F32R, kind="ExternalInput")
    cosq_d = nc.dram_tensor("cosq", [128, N], BF16, kind="ExternalInput")
    sinq_d = nc.dram_tensor("sinq", [128, N], BF16, kind="ExternalInput")
    if shared_tabs:
        cosk_d, sink_d = None, None
    else:
        cosk_d = nc.dram_tensor("cosk", [128, N], BF16, kind="ExternalInput")
        sink_d = nc.dram_tensor("sink", [128, N], BF16, kind="ExternalInput")
    y_d = nc.dram_tensor("y", [N, D], F32, kind="ExternalOutput")

    with tile.TileContext(nc) as tc:
        with (
            tc.tile_pool(name="const", bufs=1) as constp,
            tc.tile_pool(name="wts", bufs=1) as wts,
            tc.tile_pool(name="persist", bufs=1) as persist,
            tc.tile_pool(name="qsbp", bufs=12) as qsbp,
            tc.tile_pool(name="sqp", bufs=4) as sqp,
            tc.tile_pool(name="rope", bufs=3) as rope,
            tc.tile_pool(name="rsp", bufs=2) as rsp,
            tc.tile_pool(name="pt", bufs=6) as ptp,
            tc.tile_pool(name="posta", bufs=2) as posta,
            tc.tile_pool(name="otq", bufs=2) as otqp,
            tc.tile_pool(name="yout", bufs=2) as yout,
        ):
            # ---- weights / constants (DMA order = need order)
            xt_sb = wts.tile([128, FC, N], BF16, tag="xt")
            xt_r = xt_d.rearrange("(c p) n -> p c n", p=128)
            nc.sync.dma_start(xt_sb[:, :, 0:512], xt_r[:, :, 0:512])
            wqk_sb = wts.tile([128, FC, 6, 128], BF16, tag="wqk")
            wqk_r = wqk_d.rearrange("p (c g m) -> p c g m", c=FC, g=6)
            nc.sync.dma_start(wqk_sb[:, :, 0, :], wqk_r[:, :, 0, :])
            nc.sync.dma_start(wqk_sb[:, :, 1:6, :], wqk_r[:, :, 1:6, :])

            sel_sb = constp.tile([128, 6, 12], BF16, tag="sel")
            nc.sync.dma_start(sel_sb[:, :, :], sel_d.rearrange("p (g r) -> p g r", g=6))
            emat_sb = constp.tile([12, 6, 128], F32R, tag="emat")
            nc.sync.dma_start(emat_sb[:, :, :], emat_d.rearrange("p (g m) -> p g m", g=6))
            perm_sb = constp.tile([128, 128], BF16, tag="perm")
            nc.sync.dma_start(perm_sb[:, :], perm_d[:, :])

            tabs = {}
            tab_srcs = [(cosq_d, "cosq"), (sinq_d, "sinq")]
            if not shared_tabs:
                tab_srcs += [(cosk_d, "cosk"), (sink_d, "sink")]
            for td, nm in tab_srcs:
                tsb = constp.tile([128, N], BF16, tag=nm, name=nm)
                nc.sync.dma_start(tsb[:, :], td[:, :])
                tabs[nm] = tsb
            if shared_tabs:
                tabs["cosk"] = tabs["cosq"]
                tabs["sink"] = tabs["sinq"]

            wv_sb = wts.tile([128, FC, HPC * HD], BF16, tag="wv")
            nc.sync.dma_start(wv_sb[:, :, :], wv_d.rearrange("(c p) n -> p c n", p=128))
            for j in range(1, NJ):
                nc.sync.dma_start(
                    xt_sb[:, :, j * 512:(j + 1) * 512], xt_r[:, :, j * 512:(j + 1) * 512]
                )
            wo_sb = wts.tile([128, 3, D], F32R, tag="wo")
            nc.sync.dma_start(wo_sb[:, :, :], wo_d.rearrange("(c p) n -> p c n", p=128))

            ones64 = constp.tile([128, 64], F32R)
            nc.vector.memset(ones64[:, :].bitcast(F32), 1.0)
            eps_t = constp.tile([128, 1], F32)
            nc.vector.memset(eps_t[:, :], EPS)
            zero_t = constp.tile([128, 1], F32)
            nc.vector.memset(zero_t[:, :], 0.0)

            qt_sb = persist.tile([128, 3, N], BF16, tag="qt")
            kt_sb = persist.tile([128, 3, N], BF16, tag="kt")
            vaug = persist.tile([128, NT, HPC, 65], BF16, tag="vaug")
            nc.vector.memset(vaug[:, :, :, 64:65], 1.0)

            # ================= phase 1: qkv + norm/rope, transposed q/k
            with (
                tc.tile_pool(name="p1w", bufs=4, space="PSUM") as p1w,
                tc.tile_pool(name="p1rsb", bufs=2, space="PSUM") as p1rsb,
                tc.tile_pool(name="p1ss", bufs=2, space="PSUM") as p1ss,
            ):
                for j in range(NJ):
                    js = j * 512
                    q_sbs = []
                    ss_ps = p1ss.tile([12, 512], F32, tag="ss")
                    for g in range(6):
                        q_ps = p1w.tile([128, 512], F32, tag="work", name=f"qps{g}")
                        for c in range(FC):
                            nc.tensor.matmul(
                                q_ps[:, :],
                                wqk_sb[:, c, g, :],
                                xt_sb[:, c, js:js + 512],
                                start=(c == 0),
                                stop=(c == FC - 1),
                            )
                        sq_sb = sqp.tile([128, 512], BF16, tag="sq", name=f"sq{g}")
                        nc.scalar.activation(sq_sb[:, :], q_ps[:, :], ACT.Square)
                        q_sb = qsbp.tile([128, 512], BF16, tag="qsb", name=f"qsb{g}")
                        nc.scalar.activation(q_sb[:, :], q_ps[:, :], ACT.Copy)
                        q_sbs.append(q_sb)
                        nc.tensor.matmul(
                            ss_ps[:, :], sel_sb[:, g, :], sq_sb[:, :],
                            start=(g == 0), stop=(g == 5),
                        )
                    # rsqrt(mean + eps) = exp(-0.5 * ln(sum/64 + eps)), all heads
                    lg_sb = rsp.tile([12, 512], F32, tag="lg")
                    nc.scalar.activation(lg_sb[:, :], ss_ps[:, :], ACT.Ln,
                                         bias=eps_t[0:12, :], scale=1.0 / HD)
                    rs_sb = rsp.tile([12, 512], F32R, tag="rs")
                    nc.scalar.activation(rs_sb[:, :], lg_sb[:, :], ACT.Exp,
                                         bias=zero_t[0:12, :], scale=-0.5)
                    for g in range(6):
                        isq = g < 3
                        gg = g if isq else g - 3
                        cosb = tabs["cosq" if isq else "cosk"]
                        sinb = tabs["sinq" if isq else "sink"]
                        rsb = p1rsb.tile([128, 512], F32, tag="rsb")
                        nc.tensor.matmul(rsb[:, :], emat_sb[:, g, :], rs_sb[:, :],
                                         start=True, stop=True)
                        qrot = p1w.tile([128, 512], F32, tag="work", name=f"qrot{g}")
                        nc.tensor.matmul(qrot[:, :], perm_sb[:, :], q_sbs[g][:, :],
                                         start=True, stop=True)
                        a_sb = rope.tile([128, 512], BF16, tag="a")
                        nc.vector.tensor_mul(a_sb[:, :], q_sbs[g][:, :],
                                             cosb[:, js:js + 512])
                        b_sb = rope.tile([128, 512], BF16, tag="b")
                        nc.vector.tensor_mul(b_sb[:, :], qrot[:, :],
                                             sinb[:, js:js + 512])
                        c_sb = rope.tile([128, 512], BF16, tag="c")
                        nc.vector.tensor_add(c_sb[:, :], a_sb[:, :], b_sb[:, :])
                        dst = qt_sb if isq else kt_sb
                        nc.vector.tensor_mul(dst[:, gg, js:js + 512], c_sb[:, :],
                                             rsb[:, :])
                    # V for the 4 seq-tiles of this chunk
                    for it in range(4):
                        i = 4 * j + it
                        v_ps = p1w.tile([128, 512], F32, tag="work", name=f"vps{it}")
                        for c in range(FC):
                            nc.tensor.matmul(
                                v_ps[:, 0:384],
                                xt_sb[:, c, i * 128:(i + 1) * 128],
                                wv_sb[:, c, :],
                                start=(c == 0),
                                stop=(c == FC - 1),
                            )
                        nc.scalar.activation(
                            vaug[:, i, :, 0:64],
                            v_ps[:, 0:384].rearrange("p (h d) -> p h d", h=HPC),
                            ACT.Copy,
                        )

            # ================= phase 2: attention per (qc, head)
            with (
                tc.tile_pool(name="workp", bufs=3, space="PSUM") as workp,
                tc.tile_pool(name="otp", bufs=2, space="PSUM") as otp,
            ):
                def emit_tail(st):
                    qs, recAr, recBr, otun, otq = st
                    for hloc in range(HPC):
                        rt, dr = (recAr, hloc) if hloc < 4 else (recBr, hloc - 4)
                        bcw = workp.tile([128, 2, 512], F32, tag="work", name="bcw")
                        bc = bcw[0:64, 0, :]
                        nc.tensor.matmul(bc, ones64[32 * dr:32 * dr + 1, :],
                                         rt[32 * dr:32 * dr + 1, :],
                                         start=True, stop=True,
                                         tile_position=(32 * dr, 0))
                        pp, hh = hloc // 2, hloc % 2
                        nc.vector.tensor_mul(
                            otq[hh * 64:(hh + 1) * 64, pp, :],
                            otun[:, hloc, :],
                            bc,
                        )
                    for qt4 in range(4):
                        q0 = qs + qt4 * 128
                        yps = workp.tile([128, 2, 512], F32, tag="work")
                        for c in range(3):
                            nc.tensor.matmul(
                                yps[:, 0, :], otq[:, c, qt4 * 128:(qt4 + 1) * 128],
                                wo_sb[:, c, 0:512],
                                start=(c == 0), stop=(c == 2),
                            )
                        for c in range(3):
                            nc.tensor.matmul(
                                yps[:, 1, 0:256], otq[:, c, qt4 * 128:(qt4 + 1) * 128],
                                wo_sb[:, c, 512:768],
                                start=(c == 0), stop=(c == 2),
                            )
                        ysb = yout.tile([128, D], F32, tag="ysb")
                        nc.vector.tensor_copy(ysb[:, 0:512], yps[:, 0, :])
                        nc.vector.tensor_copy(ysb[:, 512:768], yps[:, 1, 0:256])
                        nc.sync.dma_start(y_d[q0:q0 + 128, :], ysb[:, :])

                pending = None
                for qc in range(4):
                    qs = qc * 512
                    recA = posta.tile([128, 512], F32, tag="recA")
                    recB = posta.tile([128, 512], F32, tag="recB")
                    lgA = posta.tile([128, 512], F32, tag="lgA")
                    lgB = posta.tile([128, 512], F32, tag="lgB")
                    recAr = posta.tile([128, 512], F32R, tag="recAr")
                    recBr = posta.tile([128, 512], F32R, tag="recBr")
                    otun = posta.tile([64, HPC, 512], F32, tag="otun")
                    otq = otqp.tile([128, 3, 512], F32R, tag="otq")
                    for pp in range(3):
                        ots = []
                        for hh in range(2):
                            otps = otp.tile([65, 512], F32, tag="ot", name=f"otps{hh}")
                            ots.append(otps)

                        def emit_av(kt, pt):
                            for hh in range(2):
                                hloc = pp * 2 + hh
                                nc.tensor.matmul(
                                    ots[hh][:, :],
                                    vaug[:, kt, hloc, :],
                                    pt[:, hh, :],
                                    start=(kt == 0), stop=(kt == NT - 1),
                                )

                        prev = None
                        for kt in range(NT):
                            slab = workp.tile([128, 2, 512], F32, tag="work")
                            for hh in range(2):
                                r0, r1 = hh * 64, hh * 64 + 64
                                nc.tensor.matmul(
                                    slab[:, hh, :],
                                    kt_sb[r0:r1, pp, kt * 128:(kt + 1) * 128],
                                    qt_sb[r0:r1, pp, qs:qs + 512],
                                    start=True, stop=True,
                                )
                            pt = ptp.tile([128, 2, 512], BF16, tag="pt")
                            if kt in DVE_KT:
                                nc.vector.tensor_scalar(
                                    pt[:, :, :].bitcast(I16), slab[:, :, :],
                                    float(A16), float(B16), ALU.mult, ALU.add,
                                )
                            else:
                                nc.scalar.activation(
                                    pt[:, :, :], slab[:, :, :], ACT.Exp,
                                    bias=zero_t[:, :], scale=SCALE,
                                )
                            # AV lags one key tile so it never heads the PE
                            # queue before its exp has finished
                            if prev is not None:
                                emit_av(*prev)
                            prev = (kt, pt)
                        emit_av(*prev)
                        for hh in range(2):
                            hloc = pp * 2 + hh
                            rt, dr = (recA, hloc) if hloc < 4 else (recB, hloc - 4)
                            nc.vector.tensor_copy(rt[32 * dr:32 * dr + 1, :],
                                                  ots[hh][64:65, :])
                            nc.vector.tensor_copy(otun[:, hloc, :],
                                                  ots[hh][0:64, :])
                        if pp == 1:
                            nc.scalar.activation(lgA[:, :], recA[:, :], ACT.Ln,
                                                 bias=zero_t[:, :])
                            nc.scalar.activation(recAr[:, :], lgA[:, :], ACT.Exp,
                                                 bias=zero_t[:, :], scale=-1.0)
                        if pp == 2:
                            nc.scalar.activation(lgB[0:64, :], recB[0:64, :], ACT.Ln,
                                                 bias=zero_t[0:64, :])
                            nc.scalar.activation(recBr[0:64, :], lgB[0:64, :],
                                                 ACT.Exp,
                                                 bias=zero_t[0:64, :], scale=-1.0)
                        if pp == 0 and pending is not None:
                            emit_tail(pending)
                            pending = None
                    pending = (qs, recAr, recBr, otun, otq)
                emit_tail(pending)

    split_multi_waits(nc)
    return nc


# ---------------------------------------------------------------- entry
def kernel(x, qkv_w, qkv_b, proj_w, proj_b, q_norm_w, k_norm_w, _trace=False):
    x = np.asarray(x, dtype=np.float32)
    qkv_w = np.asarray(qkv_w, dtype=np.float32)
    qkv_b = np.asarray(qkv_b, dtype=np.float32)
    proj_w = np.asarray(proj_w, dtype=np.float32)
    proj_b = np.asarray(proj_b, dtype=np.float32)
    q_norm_w = np.asarray(q_norm_w, dtype=np.float32)
    k_norm_w = np.asarray(k_norm_w, dtype=np.float32)

    use_bias = bool(np.any(qkv_b != 0.0))
    shared_tabs = bool(np.array_equal(q_norm_w, k_norm_w))
    key = (use_bias, shared_tabs)
    if key not in _CACHE:
        _CACHE[key] = build_nc(use_bias, shared_tabs)
    nc = _CACHE[key]
    FC = 7 if use_bias else 6

    cosq, sinq = _rope_tables_t(q_norm_w)
    cosk, sink = _rope_tables_t(k_norm_w)
    perm, sel, emat = _const_mats()

    in_maps = []
    for core in range(8):
        b, hg = core // 2, core % 2
        h0 = hg * HPC
        cols = slice(h0 * HD, (h0 + HPC) * HD)
        xt = np.ascontiguousarray(x[b].T)                       # [768, N]
        wq = qkv_w[:, cols]                                     # [768, 384]
        wk = qkv_w[:, D:][:, cols]
        wv = qkv_w[:, 2 * D:][:, cols]
        if use_bias:
            pad = np.zeros((128, N), np.float32)
            pad[0, :] = 1.0
            xt = np.concatenate([xt, pad], axis=0)
            wpad = np.zeros((128, HPC * HD), np.float32)
            wqb = np.concatenate([wq, wpad], axis=0)
            wkb = np.concatenate([wk, wpad], axis=0)
            wvb = np.concatenate([wv, wpad], axis=0)
            wqb[D, :] = qkv_b[cols]
            wkb[D, :] = qkv_b[D:][cols]
            wvb[D, :] = qkv_b[2 * D:][cols]
            wq, wk, wv = wqb, wkb, wvb
        # wqk: [p, c, g, m]; g 0-2 = q mtiles, 3-5 = k mtiles
        wqk = np.zeros((128, FC, 6, 128), np.float32)
        for g in range(3):
            wqk[:, :, g, :] = wq[:, g * 128:(g + 1) * 128] \
                .reshape(FC, 128, 128).transpose(1, 0, 2)
            wqk[:, :, 3 + g, :] = wk[:, g * 128:(g + 1) * 128] \
                .reshape(FC, 128, 128).transpose(1, 0, 2)
        wo = proj_w[h0 * HD:(h0 + HPC) * HD, :]
        im = {
            "xt": to_bf16(xt),
            "wqk": to_bf16(wqk.reshape(128, FC * 6 * 128)),
            "wv": to_bf16(wv),
            "wo": round_f32r(wo),
            "perm": perm, "sel": sel, "emat": emat,
            "cosq": cosq, "sinq": sinq,
        }
        if not shared_tabs:
            im["cosk"] = cosk
            im["sink"] = sink
        in_maps.append(im)

    res = run_bass_kernel_spmd(nc, in_maps, core_ids=list(range(8)),
                               trace=_trace or KERNEL_TRACE)
    kernel._last = res

    y = np.empty((B, N, D), dtype=np.float32)
    for b in range(B):
        y[b] = res.results[2 * b]["y"] + res.results[2 * b + 1]["y"] + proj_b[None, :]
    return y


# revision 32
# speedup vs baseline: 1.1883x; 1.1883x over previous
"""Multi-head attention (RMSNorm-QK + RoPE) Trainium2 Bass kernel, v2.

Sharding: 8 cores = 4 batches x 2 head-groups (6 heads each).
Each core computes, for its (batch, head-group):
  qkv = x @ Wqkv_slice, rmsnorm+rope on q/k, softmax(q k^T/8) v,
  y_partial = attn_out @ Wproj_rows.
Host sums the two partial y's per batch and adds proj bias.

v2 structure (vs v1):
- Phase 1 produces Q^T/K^T directly (W stationary, x^T moving) -- no PE
  transposes.  RMSNorm sum-of-squares via selector matmuls (partition
  reduce on PE), rsqrt via ScalarE ln/exp, per-seq broadcast via an
  expander matmul.  RoPE rotate-half via a permutation matmul; the
  sign and norm weight are folded into the host cos/sin tables.
- PSUM->SBUF evacuations (squares, copies, V) on ScalarE, elementwise
  rope combines on DVE in bf16.
- Softmax denominators: reciprocal_approx_fast per head (off the
  critical path), replacing full-rate RECIPROCAL + gather copies.
- QKV matmuls in bf16 (x, Wqkv); attention/proj in f32r/bf16.
"""

import sys

for _p in ("/opt/trn_rl_repo", "/root/.axon_site/_ro/trn_rl_repo"):
    if _p not in sys.path:
        sys.path.insert(0, _p)

import numpy as np
import ml_dtypes

import bass_rust
import concourse.bass as bass
import concourse.mybir as mybir
import concourse.tile as tile
from concourse.bass_utils import run_bass_kernel_spmd

# Problem constants (hardcoded per contract)
B, N, D = 4, 2048, 768
H, HD = 12, 64
HPC = 6              # heads per core
NT = N // 128        # 16 seq tiles
NJ = N // 512        # 4 seq chunks
EPS = 1e-6
THETA = 10000.0
SCALE = HD ** -0.5   # 0.125

F32 = mybir.dt.float32
F32R = mybir.dt.float32r
BF16 = mybir.dt.bfloat16
I16 = mybir.dt.int16
ACT = mybir.ActivationFunctionType
ALU = mybir.AluOpType

# Schraudolph exp in bf16 bit-space: i16 = round(s*A16 + B16); bitcast bf16
# approximates exp(SCALE*s).  Verified bit-exact vs host model on HW.
# Key tiles in DVE_KT run their exp on the DVE this way; the rest use
# ScalarE exp.  Softmax renormalization cancels part of the interp error.
A16 = SCALE * 128.0 / np.log(2.0)
B16 = 127.0 * 128.0 - 366393.0 / (1 << 16)
DVE_KT = (2, 5, 8, 11, 14)

KERNEL_TRACE = False
_CACHE = {}


# ---------------------------------------------------------------- wait split
_ctr = [0]


def _mk_nop(engine, waits=None, updates=None):
    _ctr[0] += 1
    si = mybir.SyncInfo(on_wait=waits or [], on_update=updates or [])
    return bass_rust.InstNoOp(
        name=f"I-waitfix-{_ctr[0]}", engine=engine, ins=[], outs=[], sync_info=si
    )


def split_multi_waits(nc):
    """This walrus build accepts only ONE sync wait/update per instruction;
    hoist extras onto adjacent same-engine NoOp carriers."""
    for fn in nc.m.functions:
        for bb in fn.blocks:
            insts = bb.instructions
            out = []
            changed = False
            for inst in insts:
                si = inst.sync_info
                if si is None:
                    out.append(inst)
                    continue
                waits = list(si.on_wait or [])
                updates = list(si.on_update or [])
                pre, post = [], []
                if len(waits) > 1:
                    for w in waits[:-1]:
                        pre.append(_mk_nop(inst.engine, waits=[w]))
                    si.on_wait = [waits[-1]]
                    changed = True
                if len(updates) > 1:
                    if inst.opcode == "DMACopy":
                        raise RuntimeError(
                            f"DMACopy {inst.name} has {len(updates)} updates"
                        )
                    for u in updates[1:]:
                        post.append(_mk_nop(inst.engine, updates=[u]))
                    si.on_update = [updates[0]]
                    changed = True
                out.extend(pre)
                out.append(inst)
                out.extend(post)
            if changed:
                insts[:] = out
    return nc


# ---------------------------------------------------------------- host utils
def round_f32r(a: np.ndarray) -> np.ndarray:
    """Round fp32 -> float32r (RNE to 11 mantissa bits), exact device match."""
    a = np.ascontiguousarray(a, dtype=np.float32)
    b = a.view(np.uint32).astype(np.uint64)
    drop = 12
    half = np.uint64(1 << (drop - 1))
    lsb = (b >> np.uint64(drop)) & np.uint64(1)
    out = (((b + half - np.uint64(1) + lsb) >> np.uint64(drop)) << np.uint64(drop))
    return out.astype(np.uint32).view(np.float32)


def to_bf16(a: np.ndarray) -> np.ndarray:
    return np.ascontiguousarray(a, dtype=np.float32).astype(ml_dtypes.bfloat16)


def _rope_tables_t(norm_w: np.ndarray):
    """Transposed sign-folded tables, duplicated over the two 64-row head
    blocks.  Per head block (d = p % 64):
      q'[d, n] = q[d, n]*cosw[d, n] + q[(d+32)%64, n]*sinw[d, n]
    with cosw[d] = cos[d]*w[d], sinw[d<32] = -sin[d]*w[d+32],
    sinw[d>=32] = sin[d]*w[d-32]."""
    inv_freq = 1.0 / (THETA ** (np.arange(0, HD, 2, dtype=np.float32) / HD))
    t = np.arange(N, dtype=np.float32)
    freqs = np.einsum("i,j->ij", t, inv_freq).astype(np.float32)
    emb = np.concatenate([freqs, freqs], axis=-1)  # [N, HD]
    cos = np.cos(emb).astype(np.float32)
    sin = np.sin(emb).astype(np.float32)
    w = norm_w.astype(np.float32)
    h = HD // 2
    cosw = (cos * w[None, :]).T.copy()            # [HD, N]
    sinw = np.empty((HD, N), np.float32)
    sinw[:h, :] = -(sin[:, :h] * w[None, h:]).T
    sinw[h:, :] = (sin[:, h:] * w[None, :h]).T
    cosd = np.concatenate([cosw, cosw], axis=0)   # [128, N]
    sind = np.concatenate([sinw, sinw], axis=0)
    return to_bf16(cosd), to_bf16(sind)


def _const_mats():
    h = HD // 2
    # perm: out[h0+d] = in[h0+(d+32)%64]  ->  P[p_src, p_out]
    perm = np.zeros((128, 128), np.float32)
    for h0 in (0, 64):
        for d in range(HD):
            perm[h0 + (d + h) % HD, h0 + d] = 1.0
    # sel: [128, 6, 12]  sq partitions -> ss rows
    sel = np.zeros((128, 6, 12), np.float32)
    for g in range(6):
        for p in range(128):
            r = (2 * g if g < 3 else 6 + 2 * (g - 3)) + (1 if p >= 64 else 0)
            sel[p, g, r] = 1.0
    # emat: [12, 6, 128]  ss rows -> broadcast blocks
    emat = np.zeros((12, 6, 128), np.float32)
    for g in range(6):
        for m in range(128):
            r = (2 * g if g < 3 else 6 + 2 * (g - 3)) + (1 if m >= 64 else 0)
            emat[r, g, m] = 1.0
    return to_bf16(perm), to_bf16(sel.reshape(128, 72)), \
        round_f32r(emat.reshape(12, 768))


# ---------------------------------------------------------------- bass build
def build_nc(use_bias: bool, shared_tabs: bool):
    FC = 7 if use_bias else 6  # feature chunks of 128 (7th = bias row)
    nc = bass.Bass()

    xt_d = nc.dram_tensor("xt", [FC * 128, N], BF16, kind="ExternalInput")
    wqk_d = nc.dram_tensor("wqk", [128, FC * 6 * 128], BF16, kind="ExternalInput")
    wv_d = nc.dram_tensor("wv", [FC * 128, HPC * HD], BF16, kind="ExternalInput")
    wo_d = nc.dram_tensor("wo", [HPC * HD, D], F32R, kind="ExternalInput")
    perm_d = nc.dram_tensor("perm", [128, 128], BF16, kind="ExternalInput")
    sel_d = nc.dram_tensor("sel", [128, 72], BF16, kind="ExternalInput")
    emat_d = nc.dram_tensor("emat", [12, 768], F32R, kind="ExternalInput")
    cosq_d = nc.dram_tensor("cosq", [128, N], BF16, kind="ExternalInput")
    sinq_d = nc.dram_tensor("sinq", [128, N], BF16, kind="ExternalInput")
    if shared_tabs:
        cosk_d, sink_d = None, None
    else:
        cosk_d = nc.dram_tensor("cosk", [128, N], BF16, kind="ExternalInput")
        sink_d = nc.dram_tensor("sink", [128, N], BF16, kind="ExternalInput")
    y_d = nc.dram_tensor("y", [N, D], F32, kind="ExternalOutput")

    with tile.TileContext(nc) as tc:
        with (
            tc.tile_pool(name="const", bufs=1) as constp,
            tc.tile_pool(name="wts", bufs=1) as wts,
            tc.tile_pool(name="persist", bufs=1) as persist,
            tc.tile_pool(name="qsbp", bufs=12) as qsbp,
            tc.tile_pool(name="sqp", bufs=4) as sqp,
            tc.tile_pool(name="rope", bufs=3) as rope,
            tc.tile_pool(name="rsp", bufs=2) as rsp,
            tc.tile_pool(name="pt", bufs=4) as ptp,
            tc.tile_pool(name="posta", bufs=2) as posta,
            tc.tile_pool(name="otq", bufs=2) as otqp,
            tc.tile_pool(name="yout", bufs=2) as yout,
        ):
            # ---- weights / constants (DMA order = need order)
            xt_sb = wts.tile([128, FC, N], BF16, tag="xt")
            xt_r = xt_d.rearrange("(c p) n -> p c n", p=128)
            nc.sync.dma_start(xt_sb[:, :, 0:512], xt_r[:, :, 0:512])
            wqk_sb = wts.tile([128, FC, 6, 128], BF16, tag="wqk")
            wqk_r = wqk_d.rearrange("p (c g m) -> p c g m", c=FC, g=6)
            nc.sync.dma_start(wqk_sb[:, :, 0, :], wqk_r[:, :, 0, :])
            nc.sync.dma_start(wqk_sb[:, :, 1:6, :], wqk_r[:, :, 1:6, :])

            sel_sb = constp.tile([128, 6, 12], BF16, tag="sel")
            nc.sync.dma_start(sel_sb[:, :, :], sel_d.rearrange("p (g r) -> p g r", g=6))
            emat_sb = constp.tile([12, 6, 128], F32R, tag="emat")
            nc.sync.dma_start(emat_sb[:, :, :], emat_d.rearrange("p (g m) -> p g m", g=6))
            perm_sb = constp.tile([128, 128], BF16, tag="perm")
            nc.sync.dma_start(perm_sb[:, :], perm_d[:, :])

            tabs = {}
            tab_srcs = [(cosq_d, "cosq"), (sinq_d, "sinq")]
            if not shared_tabs:
                tab_srcs += [(cosk_d, "cosk"), (sink_d, "sink")]
            for td, nm in tab_srcs:
                tsb = constp.tile([128, N], BF16, tag=nm, name=nm)
                nc.sync.dma_start(tsb[:, :], td[:, :])
                tabs[nm] = tsb
            if shared_tabs:
                tabs["cosk"] = tabs["cosq"]
                tabs["sink"] = tabs["sinq"]

            wv_sb = wts.tile([128, FC, HPC * HD], BF16, tag="wv")
            nc.sync.dma_start(wv_sb[:, :, :], wv_d.rearrange("(c p) n -> p c n", p=128))
            for j in range(1, NJ):
                nc.sync.dma_start(
                    xt_sb[:, :, j * 512:(j + 1) * 512], xt_r[:, :, j * 512:(j + 1) * 512]
                )
            wo_sb = wts.tile([128, 3, D], F32R, tag="wo")
            nc.sync.dma_start(wo_sb[:, :, :], wo_d.rearrange("(c p) n -> p c n", p=128))

            ones64 = constp.tile([128, 64], F32R)
            nc.vector.memset(ones64[:, :].bitcast(F32), 1.0)
            eps_t = constp.tile([128, 1], F32)
            nc.vector.memset(eps_t[:, :], EPS)
            zero_t = constp.tile([128, 1], F32)
            nc.vector.memset(zero_t[:, :], 0.0)

            qt_sb = persist.tile([128, 3, N], BF16, tag="qt")
            kt_sb = persist.tile([128, 3, N], BF16, tag="kt")
            vaug = persist.tile([128, NT, HPC, 65], BF16, tag="vaug")
            nc.vector.memset(vaug[:, :, :, 64:65], 1.0)

            # ================= phase 1: qkv + norm/rope, transposed q/k
            with (
                tc.tile_pool(name="p1w", bufs=4, space="PSUM") as p1w,
                tc.tile_pool(name="p1rsb", bufs=2, space="PSUM") as p1rsb,
                tc.tile_pool(name="p1ss", bufs=2, space="PSUM") as p1ss,
            ):
                for j in range(NJ):
                    js = j * 512
                    q_sbs = []
                    ss_ps = p1ss.tile([12, 512], F32, tag="ss")
                    for g in range(6):
                        q_ps = p1w.tile([128, 512], F32, tag="work", name=f"qps{g}")
                        for c in range(FC):
                            nc.tensor.matmul(
                                q_ps[:, :],
                                wqk_sb[:, c, g, :],
                                xt_sb[:, c, js:js + 512],
                                start=(c == 0),
                                stop=(c == FC - 1),
                            )
                        sq_sb = sqp.tile([128, 512], BF16, tag="sq", name=f"sq{g}")
                        nc.scalar.activation(sq_sb[:, :], q_ps[:, :], ACT.Square)
                        q_sb = qsbp.tile([128, 512], BF16, tag="qsb", name=f"qsb{g}")
                        nc.scalar.activation(q_sb[:, :], q_ps[:, :], ACT.Copy)
                        q_sbs.append(q_sb)
                        nc.tensor.matmul(
                            ss_ps[:, :], sel_sb[:, g, :], sq_sb[:, :],
                            start=(g == 0), stop=(g == 5),
                        )
                    # rsqrt(mean + eps) = exp(-0.5 * ln(sum/64 + eps)), all heads
                    lg_sb = rsp.tile([12, 512], F32, tag="lg")
                    nc.scalar.activation(lg_sb[:, :], ss_ps[:, :], ACT.Ln,
                                         bias=eps_t[0:12, :], scale=1.0 / HD)
                    rs_sb = rsp.tile([12, 512], F32R, tag="rs")
                    nc.scalar.activation(rs_sb[:, :], lg_sb[:, :], ACT.Exp,
                                         bias=zero_t[0:12, :], scale=-0.5)
                    for g in range(6):
                        isq = g < 3
                        gg = g if isq else g - 3
                        cosb = tabs["cosq" if isq else "cosk"]
                        sinb = tabs["sinq" if isq else "sink"]
                        rsb = p1rsb.tile([128, 512], F32, tag="rsb")
                        nc.tensor.matmul(rsb[:, :], emat_sb[:, g, :], rs_sb[:, :],
                                         start=True, stop=True)
                        qrot = p1w.tile([128, 512], F32, tag="work", name=f"qrot{g}")
                        nc.tensor.matmul(qrot[:, :], perm_sb[:, :], q_sbs[g][:, :],
                                         start=True, stop=True)
                        a_sb = rope.tile([128, 512], BF16, tag="a")
                        nc.vector.tensor_mul(a_sb[:, :], q_sbs[g][:, :],
                                             cosb[:, js:js + 512])
                        b_sb = rope.tile([128, 512], BF16, tag="b")
                        nc.vector.tensor_mul(b_sb[:, :], qrot[:, :],
                                             sinb[:, js:js + 512])
                        c_sb = rope.tile([128, 512], BF16, tag="c")
                        nc.vector.tensor_add(c_sb[:, :], a_sb[:, :], b_sb[:, :])
                        dst = qt_sb if isq else kt_sb
                        nc.vector.tensor_mul(dst[:, gg, js:js + 512], c_sb[:, :],
                                             rsb[:, :])
                    # V for the 4 seq-tiles of this chunk
                    for it in range(4):
                        i = 4 * j + it
                        v_ps = p1w.tile([128, 512], F32, tag="work", name=f"vps{it}")
                        for c in range(FC):
                            nc.tensor.matmul(
                                v_ps[:, 0:384],
                                xt_sb[:, c, i * 128:(i + 1) * 128],
                                wv_sb[:, c, :],
                                start=(c == 0),
                                stop=(c == FC - 1),
                            )
                        nc.scalar.activation(
                            vaug[:, i, :, 0:64],
                            v_ps[:, 0:384].rearrange("p (h d) -> p h d", h=HPC),
                            ACT.Copy,
                        )

            # ================= phase 2: attention per (qc, head)
            with (
                tc.tile_pool(name="workp", bufs=3, space="PSUM") as workp,
                tc.tile_pool(name="otp", bufs=2, space="PSUM") as otp,
            ):
                def emit_tail(st):
                    qs, recAr, recBr, otun, otq = st
                    for hloc in range(HPC):
                        rt, dr = (recAr, hloc) if hloc < 4 else (recBr, hloc - 4)
                        bcw = workp.tile([128, 2, 512], F32, tag="work", name="bcw")
                        bc = bcw[0:64, 0, :]
                        nc.tensor.matmul(bc, ones64[32 * dr:32 * dr + 1, :],
                                         rt[32 * dr:32 * dr + 1, :],
                                         start=True, stop=True,
                                         tile_position=(32 * dr, 0))
                        pp, hh = hloc // 2, hloc % 2
                        nc.vector.tensor_mul(
                            otq[hh * 64:(hh + 1) * 64, pp, :],
                            otun[:, hloc, :],
                            bc,
                        )
                    for qt4 in range(4):
                        q0 = qs + qt4 * 128
                        yps = workp.tile([128, 2, 512], F32, tag="work")
                        for c in range(3):
                            nc.tensor.matmul(
                                yps[:, 0, :], otq[:, c, qt4 * 128:(qt4 + 1) * 128],
                                wo_sb[:, c, 0:512],
                                start=(c == 0), stop=(c == 2),
                            )
                        for c in range(3):
                            nc.tensor.matmul(
                                yps[:, 1, 0:256], otq[:, c, qt4 * 128:(qt4 + 1) * 128],
                                wo_sb[:, c, 512:768],
                                start=(c == 0), stop=(c == 2),
                            )
                        ysb = yout.tile([128, D], F32, tag="ysb")
                        nc.vector.tensor_copy(ysb[:, 0:512], yps[:, 0, :])
                        nc.vector.tensor_copy(ysb[:, 512:768], yps[:, 1, 0:256])
                        nc.sync.dma_start(y_d[q0:q0 + 128, :], ysb[:, :])

                pending = None
                for qc in range(4):
                    qs = qc * 512
                    recA = posta.tile([128, 512], F32, tag="recA")
                    recB = posta.tile([128, 512], F32, tag="recB")
                    lgA = posta.tile([128, 512], F32, tag="lgA")
                    lgB = posta.tile([128, 512], F32, tag="lgB")
                    recAr = posta.tile([128, 512], F32R, tag="recAr")
                    recBr = posta.tile([128, 512], F32R, tag="recBr")
                    otun = posta.tile([64, HPC, 512], F32, tag="otun")
                    otq = otqp.tile([128, 3, 512], F32R, tag="otq")
                    for pp in range(3):
                        ots = []
                        for hh in range(2):
                            otps = otp.tile([65, 512], F32, tag="ot", name=f"otps{hh}")
                            ots.append(otps)

                        def emit_av(kt, pt):
                            for hh in range(2):
                                hloc = pp * 2 + hh
                                nc.tensor.matmul(
                                    ots[hh][:, :],
                                    vaug[:, kt, hloc, :],
                                    pt[:, hh, :],
                                    start=(kt == 0), stop=(kt == NT - 1),
                                )

                        pend = []
                        for kt in range(NT):
                            slab = workp.tile([128, 2, 512], F32, tag="work")
                            for hh in range(2):
                                r0, r1 = hh * 64, hh * 64 + 64
                                nc.tensor.matmul(
                                    slab[:, hh, :],
                                    kt_sb[r0:r1, pp, kt * 128:(kt + 1) * 128],
                                    qt_sb[r0:r1, pp, qs:qs + 512],
                                    start=True, stop=True,
                                )
                            pt = ptp.tile([128, 2, 512], BF16, tag="pt")
                            if kt in DVE_KT:
                                nc.vector.tensor_scalar(
                                    pt[:, :, :].bitcast(I16), slab[:, :, :],
                                    float(A16), float(B16), ALU.mult, ALU.add,
                                )
                            else:
                                nc.scalar.activation(
                                    pt[:, :, :], slab[:, :, :], ACT.Exp,
                                    bias=zero_t[:, :], scale=SCALE,
                                )
                            # AV lags two key tiles so it never heads the PE
                            # queue before its exp has finished
                            if len(pend) == 2:
                                emit_av(*pend.pop(0))
                            pend.append((kt, pt))
                        for st in pend:
                            emit_av(*st)
                        for hh in range(2):
                            hloc = pp * 2 + hh
                            rt, dr = (recA, hloc) if hloc < 4 else (recB, hloc - 4)
                            nc.vector.tensor_copy(rt[32 * dr:32 * dr + 1, :],
                                                  ots[hh][64:65, :])
                            nc.scalar.activation(otun[:, hloc, :],
                                                 ots[hh][0:64, :], ACT.Copy)
                        if pp == 1:
                            nc.scalar.activation(lgA[:, :], recA[:, :], ACT.Ln,
                                                 bias=zero_t[:, :])
                            nc.scalar.activation(recAr[:, :], lgA[:, :], ACT.Exp,
                                                 bias=zero_t[:, :], scale=-1.0)
                        if pp == 2:
                            nc.scalar.activation(lgB[0:64, :], recB[0:64, :], ACT.Ln,
                                                 bias=zero_t[0:64, :])
                            nc.scalar.activation(recBr[0:64, :], lgB[0:64, :],
                                                 ACT.Exp,
                                                 bias=zero_t[0:64, :], scale=-1.0)
                        if pp == 0 and pending is not None:
                            emit_tail(pending)
                            pending = None
                    pending = (qs, recAr, recBr, otun, otq)
                emit_tail(pending)

    split_multi_waits(nc)
    return nc


# ---------------------------------------------------------------- entry
def kernel(x, qkv_w, qkv_b, proj_w, proj_b, q_norm_w, k_norm_w, _trace=False):
    x = np.asarray(x, dtype=np.float32)
    qkv_w = np.asarray(qkv_w, dtype=np.float32)
    qkv_b = np.asarray(qkv_b, dtype=np.float32)
    proj_w = np.asarray(proj_w, dtype=np.float32)
    proj_b = np.asarray(proj_b, dtype=np.float32)
    q_norm_w = np.asarray(q_norm_w, dtype=np.float32)
    k_norm_w = np.asarray(k_norm_w, dtype=np.float32)

    use_bias = bool(np.any(qkv_b != 0.0))
    shared_tabs = bool(np.array_equal(q_norm_w, k_norm_w))
    key = (use_bias, shared_tabs)
    if key not in _CACHE:
        _CACHE[key] = build_nc(use_bias, shared_tabs)
    nc = _CACHE[key]
    FC = 7 if use_bias else 6

    cosq, sinq = _rope_tables_t(q_norm_w)
    cosk, sink = _rope_tables_t(k_norm_w)
    perm, sel, emat = _const_mats()

    in_maps = []
    for core in range(8):
        b, hg = core // 2, core % 2
        h0 = hg * HPC
        cols = slice(h0 * HD, (h0 + HPC) * HD)
        xt = np.ascontiguousarray(x[b].T)                       # [768, N]
        wq = qkv_w[:, cols]                                     # [768, 384]
        wk = qkv_w[:, D:][:, cols]
        wv = qkv_w[:, 2 * D:][:, cols]
        if use_bias:
            pad = np.zeros((128, N), np.float32)
            pad[0, :] = 1.0
            xt = np.concatenate([xt, pad], axis=0)
            wpad = np.zeros((128, HPC * HD), np.float32)
            wqb = np.concatenate([wq, wpad], axis=0)
            wkb = np.concatenate([wk, wpad], axis=0)
            wvb = np.concatenate([wv, wpad], axis=0)
            wqb[D, :] = qkv_b[cols]
            wkb[D, :] = qkv_b[D:][cols]
            wvb[D, :] = qkv_b[2 * D:][cols]
            wq, wk, wv = wqb, wkb, wvb
        # wqk: [p, c, g, m]; g 0-2 = q mtiles, 3-5 = k mtiles
        wqk = np.zeros((128, FC, 6, 128), np.float32)
        for g in range(3):
            wqk[:, :, g, :] = wq[:, g * 128:(g + 1) * 128] \
                .reshape(FC, 128, 128).transpose(1, 0, 2)
            wqk[:, :, 3 + g, :] = wk[:, g * 128:(g + 1) * 128] \
                .reshape(FC, 128, 128).transpose(1, 0, 2)
        wo = proj_w[h0 * HD:(h0 + HPC) * HD, :]
        im = {
            "xt": to_bf16(xt),
            "wqk": to_bf16(wqk.reshape(128, FC * 6 * 128)),
            "wv": to_bf16(wv),
            "wo": round_f32r(wo),
            "perm": perm, "sel": sel, "emat": emat,
            "cosq": cosq, "sinq": sinq,
        }
        if not shared_tabs:
            im["cosk"] = cosk
            im["sink"] = sink
        in_maps.append(im)

    res = run_bass_kernel_spmd(nc, in_maps, core_ids=list(range(8)),
                               trace=_trace or KERNEL_TRACE)
    kernel._last = res

    y = np.empty((B, N, D), dtype=np.float32)
    for b in range(B):
        y[b] = res.results[2 * b]["y"] + res.results[2 * b + 1]["y"] + proj_b[None, :]
    return y
